# revision 12
# baseline (speedup 1.0000x reference)
"""Trainium2 Bass kernel for nn_Attention: tanh-scored softmax attention pooling.

reference:
    cat = concat([query, embeddings], axis=2)            # (B, L, Q+E)
    h = tanh(cat @ W_attn.T + b_attn)                    # (B, L, E)
    scores = h @ v_w                                     # (B, L)
    w = softmax(scores, axis=1)                          # (B, L)
    applied = (w[:, None, :] @ embeddings)               # (B, 1, E)
    returns (applied, w[:, None, :])

Sharding: data-parallel over batch, 4 batches per core on 8 cores.
Weights (W_attn, b_attn, v_w) replicated.

Per-core dataflow (fp32 data, matmuls in fp32r at 1 cyc/row):
  - load q/emb natural tiles [128 l, 512 f] (f32r-typed, same bits as f32)
  - PE-transpose 128x128 blocks into catT chunks (f32r, 1.5 cyc/row), DVE-copy to sbuf
  - h[l,e] accumulated in psum over 8 f-chunks (lhsT = catT chunk, rhs = W_T chunk)
  - DVE adds bias (b_attn broadcast tile), ACT tanh
  - DVE scalar_tensor_tensor(h * v_bcast, accum=sum over e) -> scores column
    (tensor_tensor_reduce is a custom-DVE op that crashes this runtime)
  - per batch: ACT exp (f32r out), PE ones-matmul partition-sum, DVE reciprocal,
    PE broadcast of 1/S, DVE scale -> weights; PE transpose (f32) -> DMA out
  - applied = sum_t exp_col_t.T @ emb_tile_t (f32r psum accum), scaled by 1/S
"""

import sys

for _p in ("/opt/trn_rl_repo",):
    if _p not in sys.path:
        sys.path.append(_p)

import numpy as np

import concourse.bass as bass
import concourse.tile as tile
from concourse import bacc, mybir
from concourse.bass_utils import run_bass_kernel_spmd
from concourse.masks import make_identity

F32 = mybir.dt.float32
F32R = mybir.dt.float32r

N_CORES = 8
B, L, E, Q = 32, 2048, 512, 512
F = E + Q                      # cat feature dim
B_PC = B // N_CORES            # batches per core
LT = L // 128                  # l-tiles per batch
FC = F // 128                  # 128-wide f chunks in cat
QC = Q // 128                  # f chunks coming from query


def build_nc(b_pc=B_PC, lt=LT, reps=1, timing_R=None):
    """timing_R: if set, embeddings/query become internal DRAM (garbage data,
    no host transfer) and the whole body runs in a hardware For_i loop of
    timing_R iterations — used only to measure per-iteration HW time."""
    nc = bacc.Bacc("TRN2", target_bir_lowering=False, debug=False)

    if timing_R is not None:
        emb_d = nc.dram_tensor("emb_int", [b_pc, lt * 128, E], F32R).ap()
        q_d = nc.dram_tensor("q_int", [b_pc, lt * 128, Q], F32R).ap()
    else:
        emb_d = nc.dram_tensor("embeddings", [b_pc, lt * 128, E], F32R, kind="ExternalInput").ap()
        q_d = nc.dram_tensor("query", [b_pc, lt * 128, Q], F32R, kind="ExternalInput").ap()
    wt_d = nc.dram_tensor("w_t", [F, E], F32R, kind="ExternalInput").ap()      # W_attn.T
    bias_d = nc.dram_tensor("bias_bc", [128, E], F32, kind="ExternalInput").ap()
    v_d = nc.dram_tensor("v_bc", [128, E], F32, kind="ExternalInput").ap()     # v_w bcast
    id_d = nc.dram_tensor("ident", [128, 128], F32R, kind="ExternalInput").ap()
    onec_d = nc.dram_tensor("ones_col", [128, 1], F32R, kind="ExternalInput").ap()
    app_d = nc.dram_tensor("applied", [b_pc, 1, E], F32, kind="ExternalOutput").ap()
    wout_d = nc.dram_tensor("weights", [b_pc, 1, lt * 128], F32, kind="ExternalOutput").ap()

    with tile.TileContext(nc) as tc:
        with (
            tc.tile_pool(name="consts", bufs=1) as consts,
            tc.tile_pool(name="embp", bufs=2 * lt) as embp,
            tc.tile_pool(name="qp", bufs=3) as qp,
            tc.tile_pool(name="ctq", bufs=3) as ctqp,
            tc.tile_pool(name="cte", bufs=3) as ctep,
            tc.tile_pool(name="hp", bufs=3) as hp,
            tc.tile_pool(name="batchp", bufs=2) as batchp,
            tc.tile_pool(name="smallp", bufs=4) as smallp,
            tc.tile_pool(name="ps_ct", bufs=4, space="PSUM") as ps_ct,
            tc.tile_pool(name="ps_h", bufs=2, space="PSUM") as ps_h,
            tc.tile_pool(name="ps_sm", bufs=1, space="PSUM") as ps_sm,
        ):
            ident_f32 = consts.tile([128, 128], F32)
            make_identity(nc, ident_f32)
            ones = consts.tile([128, 128], F32)
            nc.vector.memset(ones, 1.0)

            ident_r = consts.tile([128, 128], F32R)
            nc.sync.dma_start(out=ident_r, in_=id_d)
            onec_sb = consts.tile([128, 1], F32R)
            nc.sync.dma_start(out=onec_sb, in_=onec_d)
            w_sb = consts.tile([128, FC, E], F32R)
            nc.sync.dma_start(out=w_sb, in_=wt_d.rearrange("(c p) e -> p c e", p=128))
            bias_sb = consts.tile([128, E], F32)
            nc.sync.dma_start(out=bias_sb, in_=bias_d)
            v_sb = consts.tile([128, E], F32)
            nc.sync.dma_start(out=v_sb, in_=v_d)

            def load_and_transpose(b, t):
                """DMA-load q/emb l-tile t, PE-transpose into catT sbuf chunks."""
                q_tile = qp.tile([128, Q], F32R)
                nc.sync.dma_start(out=q_tile, in_=q_d[b, t * 128:(t + 1) * 128, :])
                e_tile = embp.tile([128, E], F32R)
                nc.sync.dma_start(out=e_tile, in_=emb_d[b, t * 128:(t + 1) * 128, :])

                pq = ps_ct.tile([128, 4 * 128], F32R, tag="psct")
                for c in range(QC):
                    nc.tensor.transpose(
                        pq[:, c * 128:(c + 1) * 128], q_tile[:, c * 128:(c + 1) * 128], ident_r
                    )
                ctq = ctqp.tile([128, 4 * 128], F32R)
                nc.vector.tensor_copy(ctq, pq)

                pe = ps_ct.tile([128, 4 * 128], F32R, tag="psct")
                for c in range(FC - QC):
                    nc.tensor.transpose(
                        pe[:, c * 128:(c + 1) * 128], e_tile[:, c * 128:(c + 1) * 128], ident_r
                    )
                cte = ctep.tile([128, 4 * 128], F32R)
                nc.scalar.copy(cte, pe)
                return e_tile, ctq, cte

            def body():
                # one-tile-ahead software pipeline: transposes/copies of tile
                # t+1 are emitted (and scheduled) before the matmuls of tile t,
                # so the PE never waits on a copy.
                staged = load_and_transpose(0, 0)
                for b in range(b_pc):
                    etiles = []
                    scores_b = batchp.tile([128, lt], F32, tag="scores")
                    for t in range(lt):
                        e_tile, ctq, cte = staged
                        etiles.append(e_tile)
                        if t + 1 < lt:
                            staged = load_and_transpose(b, t + 1)
                        elif b + 1 < b_pc:
                            staged = load_and_transpose(b + 1, 0)

                        ph = ps_h.tile([128, E], F32)
                        for k in range(FC):
                            src = ctq if k < QC else cte
                            c = k if k < QC else k - QC
                            nc.tensor.matmul(
                                ph,
                                src[:, c * 128:(c + 1) * 128],
                                w_sb[:, k, :],
                                start=(k == 0),
                                stop=(k == FC - 1),
                            )

                        hb = hp.tile([128, E], F32, tag="hb")
                        nc.vector.tensor_add(hb, ph, bias_sb)
                        h_sb = hp.tile([128, E], F32, tag="h")
                        nc.scalar.activation(h_sb, hb, mybir.ActivationFunctionType.Tanh)
                        hv = hp.tile([128, E], F32, tag="hv")
                        nc.vector.scalar_tensor_tensor(
                            out=hv,
                            in0=h_sb,
                            scalar=1.0,
                            in1=v_sb,
                            op0=mybir.AluOpType.mult,
                            op1=mybir.AluOpType.mult,
                            accum_out=scores_b[:, t:t + 1],
                        )

                    # --- batch tail: softmax + outputs ---
                    exp_b = batchp.tile([128, lt], F32R, tag="exp")
                    nc.scalar.activation(exp_b, scores_b, mybir.ActivationFunctionType.Exp)

                    ps_s = ps_sm.tile([1, lt], F32, tag="pssm")
                    nc.tensor.matmul(ps_s, onec_sb, exp_b)  # partition sum -> [1, lt]
                    s_sb = smallp.tile([1, lt], F32, tag="s")
                    nc.vector.tensor_copy(s_sb, ps_s)
                    tot = smallp.tile([1, 1], F32, tag="tot")
                    nc.vector.reduce_sum(tot, s_sb, axis=mybir.AxisListType.X)
                    rcp = smallp.tile([1, 1], F32, tag="rcp")
                    nc.vector.reciprocal(rcp, tot)

                    ps_rb = ps_sm.tile([128, 1], F32, tag="pssm")
                    nc.tensor.matmul(ps_rb, ones[0:1, :], rcp)  # bcast 1/S to 128 parts
                    rb_sb = smallp.tile([128, 1], F32, tag="rb")
                    nc.vector.tensor_copy(rb_sb, ps_rb)

                    wgt = batchp.tile([128, lt], F32, tag="wgt")
                    nc.vector.tensor_scalar_mul(wgt, exp_b.bitcast(F32), rb_sb)
                    ps_wt = ps_sm.tile([lt, 128], F32, tag="pssm")
                    nc.tensor.transpose(ps_wt, wgt, ident_f32)
                    wt_sb = smallp.tile([lt, 128], F32, tag="wT")
                    nc.vector.tensor_copy(wt_sb, ps_wt)
                    nc.sync.dma_start(
                        out=wout_d[b, 0, :].rearrange("(t p) -> t p", p=128), in_=wt_sb
                    )

                    ps_a = ps_sm.tile([1, E], F32, tag="psa")
                    for t in range(lt):
                        nc.tensor.matmul(
                            ps_a,
                            exp_b[:, t:t + 1],
                            etiles[t],
                            start=(t == 0),
                            stop=(t == lt - 1),
                        )
                    app_sb = smallp.tile([1, E], F32, tag="app")
                    nc.vector.tensor_scalar_mul(app_sb, ps_a, rcp)
                    nc.sync.dma_start(out=app_d[b, 0:1, :], in_=app_sb)

            if timing_R is not None:
                with tc.For_i(0, timing_R, 1):
                    body()
            else:
                for _ in range(reps):
                    body()

    nc.finalize()
    return nc


_NC_CACHE = {}


def _get_nc(b_pc=B_PC, lt=LT, reps=1, timing_R=None):
    key = (b_pc, lt, reps, timing_R)
    if key not in _NC_CACHE:
        _NC_CACHE[key] = build_nc(*key)
    return _NC_CACHE[key]


def _prep_shared(W_attn, b_attn, v_w):
    return {
        "w_t": np.ascontiguousarray(W_attn.T),
        "bias_bc": np.ascontiguousarray(np.broadcast_to(b_attn, (128, E))),
        "v_bc": np.ascontiguousarray(np.broadcast_to(v_w, (128, E))),
        "ident": np.eye(128, dtype=np.float32),
        "ones_col": np.ones((128, 1), dtype=np.float32),
    }


def kernel(embeddings, query, W_attn, b_attn, v_w):
    embeddings = np.asarray(embeddings, dtype=np.float32)
    query = np.asarray(query, dtype=np.float32)
    shared = _prep_shared(np.asarray(W_attn, np.float32), np.asarray(b_attn, np.float32),
                          np.asarray(v_w, np.float32))
    nc = _get_nc()
    in_maps = []
    for i in range(N_CORES):
        sl = slice(i * B_PC, (i + 1) * B_PC)
        in_maps.append({"embeddings": embeddings[sl], "query": query[sl], **shared})
    res = run_bass_kernel_spmd(nc, in_maps, list(range(N_CORES)))
    applied = np.concatenate([res.results[i]["applied"] for i in range(N_CORES)], axis=0)
    weights = np.concatenate([res.results[i]["weights"] for i in range(N_CORES)], axis=0)
    return applied, weights


# revision 18
# speedup vs baseline: 1.4577x; 1.4577x over previous
"""Trainium2 Bass kernel for nn_Attention: tanh-scored softmax attention pooling.

reference:
    cat = concat([query, embeddings], axis=2)            # (B, L, Q+E)
    h = tanh(cat @ W_attn.T + b_attn)                    # (B, L, E)
    scores = h @ v_w                                     # (B, L)
    w = softmax(scores, axis=1)                          # (B, L)
    applied = (w[:, None, :] @ embeddings)               # (B, 1, E)
    returns (applied, w[:, None, :])

Sharding: data-parallel over batch, 4 batches per core on 8 cores.
Weights (W_attn, b_attn, v_w) replicated.

Per-core dataflow (fp16 PE inputs, fp32 accumulation/everything else):
  - q/emb cast to fp16 on host; DMA fp16 natural tiles [128 l, 512 f]
  - PE-transpose 128x128 fp16 blocks into catT chunks, DVE/ACT-copy to sbuf
  - h[l,e] accumulated in psum over 8 f-chunks (lhsT = catT chunk, rhs = W_T chunk)
  - DVE adds bias (b_attn broadcast tile), ACT tanh
  - DVE scalar_tensor_tensor(h * v_bcast, accum=sum over e) -> scores column
    (tensor_tensor_reduce is a custom-DVE op that crashes this runtime)
  - per batch: ACT exp (fp16 out), PE ones-matmul partition-sum, DVE reciprocal,
    PE broadcast of 1/S, DVE scale -> weights; PE transpose (f32) -> DMA out
  - applied = sum_t exp_col_t.T @ emb16_tile_t (fp16 in, fp32 psum), scaled by 1/S
"""

import sys

for _p in ("/opt/trn_rl_repo",):
    if _p not in sys.path:
        sys.path.append(_p)

import numpy as np

import concourse.bass as bass
import concourse.tile as tile
from concourse import bacc, mybir
from concourse.bass_utils import run_bass_kernel_spmd
from concourse.masks import make_identity

F32 = mybir.dt.float32
F32R = mybir.dt.float32r
F16 = mybir.dt.float16

N_CORES = 8
B, L, E, Q = 32, 2048, 512, 512
F = E + Q                      # cat feature dim
B_PC = B // N_CORES            # batches per core
LT = L // 128                  # l-tiles per batch
FC = F // 128                  # 128-wide f chunks in cat
QC = Q // 128                  # f chunks coming from query


def build_nc(b_pc=B_PC, lt=LT, reps=1, timing_R=None):
    """timing_R: if set, embeddings/query become internal DRAM (garbage data,
    no host transfer) and the whole body runs in a hardware For_i loop of
    timing_R iterations — used only to measure per-iteration HW time."""
    nc = bacc.Bacc("TRN2", target_bir_lowering=False, debug=False)

    if timing_R is not None:
        emb_d = nc.dram_tensor("emb_int", [b_pc, lt * 128, E], F16).ap()
        q_d = nc.dram_tensor("q_int", [b_pc, lt * 128, Q], F16).ap()
    else:
        emb_d = nc.dram_tensor("embeddings", [b_pc, lt * 128, E], F16, kind="ExternalInput").ap()
        q_d = nc.dram_tensor("query", [b_pc, lt * 128, Q], F16, kind="ExternalInput").ap()
    wt_d = nc.dram_tensor("w_t", [F, E], F16, kind="ExternalInput").ap()      # W_attn.T
    bias_d = nc.dram_tensor("bias_bc", [128, E], F32, kind="ExternalInput").ap()
    v_d = nc.dram_tensor("v_bc", [128, E], F32, kind="ExternalInput").ap()     # v_w bcast
    id_d = nc.dram_tensor("ident", [128, 128], F16, kind="ExternalInput").ap()
    onec_d = nc.dram_tensor("ones_col", [128, 1], F16, kind="ExternalInput").ap()
    app_d = nc.dram_tensor("applied", [b_pc, 1, E], F32, kind="ExternalOutput").ap()
    wout_d = nc.dram_tensor("weights", [b_pc, 1, lt * 128], F32, kind="ExternalOutput").ap()

    with tile.TileContext(nc) as tc:
        with (
            tc.tile_pool(name="consts", bufs=1) as consts,
            tc.tile_pool(name="embp", bufs=2 * lt) as embp,
            tc.tile_pool(name="qp", bufs=3) as qp,
            tc.tile_pool(name="ctq", bufs=3) as ctqp,
            tc.tile_pool(name="cte", bufs=3) as ctep,
            tc.tile_pool(name="hp", bufs=3) as hp,
            tc.tile_pool(name="batchp", bufs=2) as batchp,
            tc.tile_pool(name="smallp", bufs=4) as smallp,
            tc.tile_pool(name="ps_ct", bufs=4, space="PSUM") as ps_ct,
            tc.tile_pool(name="ps_h", bufs=2, space="PSUM") as ps_h,
            tc.tile_pool(name="ps_sm", bufs=1, space="PSUM") as ps_sm,
        ):
            ident_f32 = consts.tile([128, 128], F32)
            make_identity(nc, ident_f32)
            ones = consts.tile([128, 128], F32)
            nc.vector.memset(ones, 1.0)

            ident_r = consts.tile([128, 128], F16)
            nc.sync.dma_start(out=ident_r, in_=id_d)
            onec_sb = consts.tile([128, 1], F16)
            nc.sync.dma_start(out=onec_sb, in_=onec_d)
            w_sb = consts.tile([128, FC, E], F16)
            nc.sync.dma_start(out=w_sb, in_=wt_d.rearrange("(c p) e -> p c e", p=128))
            bias_sb = consts.tile([128, E], F32)
            nc.sync.dma_start(out=bias_sb, in_=bias_d)
            v_sb = consts.tile([128, E], F32)
            nc.sync.dma_start(out=v_sb, in_=v_d)

            def load_and_transpose(b, t):
                """DMA-load q/emb l-tile t (f32), cast to fp16, PE-transpose
                into catT sbuf chunks; keep the fp16 emb tile for applied."""
                q16 = qp.tile([128, Q], F16, tag="q16")
                nc.sync.dma_start(out=q16, in_=q_d[b, t * 128:(t + 1) * 128, :])
                e16 = embp.tile([128, E], F16)
                nc.sync.dma_start(out=e16, in_=emb_d[b, t * 128:(t + 1) * 128, :])

                pq = ps_ct.tile([128, 4 * 128], F16, tag="psct")
                for c in range(QC):
                    nc.tensor.transpose(
                        pq[:, c * 128:(c + 1) * 128], q16[:, c * 128:(c + 1) * 128], ident_r
                    )
                ctq = ctqp.tile([128, 4 * 128], F16)
                nc.vector.tensor_copy(ctq, pq)

                pe = ps_ct.tile([128, 4 * 128], F16, tag="psct")
                for c in range(FC - QC):
                    nc.tensor.transpose(
                        pe[:, c * 128:(c + 1) * 128], e16[:, c * 128:(c + 1) * 128], ident_r
                    )
                cte = ctep.tile([128, 4 * 128], F16)
                nc.scalar.copy(cte, pe)
                return e16, ctq, cte

            def body():
                # one-tile-ahead software pipeline: transposes/copies of tile
                # t+1 are emitted (and scheduled) before the matmuls of tile t,
                # so the PE never waits on a copy.
                staged = load_and_transpose(0, 0)
                for b in range(b_pc):
                    etiles = []
                    scores_b = batchp.tile([128, lt], F32, tag="scores")
                    for t in range(lt):
                        e_tile, ctq, cte = staged
                        etiles.append(e_tile)
                        if t + 1 < lt:
                            staged = load_and_transpose(b, t + 1)
                        elif b + 1 < b_pc:
                            staged = load_and_transpose(b + 1, 0)

                        ph = ps_h.tile([128, E], F32)
                        for k in range(FC):
                            src = ctq if k < QC else cte
                            c = k if k < QC else k - QC
                            nc.tensor.matmul(
                                ph,
                                src[:, c * 128:(c + 1) * 128],
                                w_sb[:, k, :],
                                start=(k == 0),
                                stop=(k == FC - 1),
                            )

                        hb = hp.tile([128, E], F32, tag="hb")
                        nc.vector.tensor_add(hb, ph, bias_sb)
                        h_sb = hp.tile([128, E], F32, tag="h")
                        nc.scalar.activation(h_sb, hb, mybir.ActivationFunctionType.Tanh)
                        hv = hp.tile([128, E], F32, tag="hv")
                        nc.vector.scalar_tensor_tensor(
                            out=hv,
                            in0=h_sb,
                            scalar=1.0,
                            in1=v_sb,
                            op0=mybir.AluOpType.mult,
                            op1=mybir.AluOpType.mult,
                            accum_out=scores_b[:, t:t + 1],
                        )

                    # --- batch tail: softmax + outputs ---
                    exp_b = batchp.tile([128, lt], F16, tag="exp")
                    nc.scalar.activation(exp_b, scores_b, mybir.ActivationFunctionType.Exp)

                    ps_s = ps_sm.tile([1, lt], F32, tag="pssm")
                    nc.tensor.matmul(ps_s, onec_sb, exp_b)  # partition sum -> [1, lt]
                    s_sb = smallp.tile([1, lt], F32, tag="s")
                    nc.vector.tensor_copy(s_sb, ps_s)
                    tot = smallp.tile([1, 1], F32, tag="tot")
                    nc.vector.reduce_sum(tot, s_sb, axis=mybir.AxisListType.X)
                    rcp = smallp.tile([1, 1], F32, tag="rcp")
                    nc.vector.reciprocal(rcp, tot)

                    ps_rb = ps_sm.tile([128, 1], F32, tag="pssm")
                    nc.tensor.matmul(ps_rb, ones[0:1, :], rcp)  # bcast 1/S to 128 parts
                    rb_sb = smallp.tile([128, 1], F32, tag="rb")
                    nc.vector.tensor_copy(rb_sb, ps_rb)

                    wgt = batchp.tile([128, lt], F32, tag="wgt")
                    nc.vector.tensor_scalar_mul(wgt, exp_b, rb_sb)
                    ps_wt = ps_sm.tile([lt, 128], F32, tag="pssm")
                    nc.tensor.transpose(ps_wt, wgt, ident_f32)
                    wt_sb = smallp.tile([lt, 128], F32, tag="wT")
                    nc.vector.tensor_copy(wt_sb, ps_wt)
                    nc.sync.dma_start(
                        out=wout_d[b, 0, :].rearrange("(t p) -> t p", p=128), in_=wt_sb
                    )

                    ps_a = ps_sm.tile([1, E], F32, tag="psa")
                    for t in range(lt):
                        nc.tensor.matmul(
                            ps_a,
                            exp_b[:, t:t + 1],
                            etiles[t],
                            start=(t == 0),
                            stop=(t == lt - 1),
                        )
                    app_sb = smallp.tile([1, E], F32, tag="app")
                    nc.vector.tensor_scalar_mul(app_sb, ps_a, rcp)
                    nc.sync.dma_start(out=app_d[b, 0:1, :], in_=app_sb)

            if timing_R is not None:
                with tc.For_i(0, timing_R, 1):
                    body()
            else:
                for _ in range(reps):
                    body()

    nc.finalize()
    return nc


_NC_CACHE = {}


def _get_nc(b_pc=B_PC, lt=LT, reps=1, timing_R=None):
    key = (b_pc, lt, reps, timing_R)
    if key not in _NC_CACHE:
        _NC_CACHE[key] = build_nc(*key)
    return _NC_CACHE[key]


def _prep_shared(W_attn, b_attn, v_w):
    return {
        "w_t": np.ascontiguousarray(W_attn.T).astype(np.float16),
        "bias_bc": np.ascontiguousarray(np.broadcast_to(b_attn, (128, E))),
        "v_bc": np.ascontiguousarray(np.broadcast_to(v_w, (128, E))),
        "ident": np.eye(128, dtype=np.float16),
        "ones_col": np.ones((128, 1), dtype=np.float16),
    }


def kernel(embeddings, query, W_attn, b_attn, v_w):
    embeddings = np.asarray(embeddings, dtype=np.float32).astype(np.float16)
    query = np.asarray(query, dtype=np.float32).astype(np.float16)
    shared = _prep_shared(np.asarray(W_attn, np.float32), np.asarray(b_attn, np.float32),
                          np.asarray(v_w, np.float32))
    nc = _get_nc()
    in_maps = []
    for i in range(N_CORES):
        sl = slice(i * B_PC, (i + 1) * B_PC)
        in_maps.append({"embeddings": embeddings[sl], "query": query[sl], **shared})
    res = run_bass_kernel_spmd(nc, in_maps, list(range(N_CORES)))
    applied = np.concatenate([res.results[i]["applied"] for i in range(N_CORES)], axis=0)
    weights = np.concatenate([res.results[i]["weights"] for i in range(N_CORES)], axis=0)
    return applied, weights


# revision 21
# speedup vs baseline: 1.6479x; 1.1305x over previous
"""Trainium2 Bass kernel for nn_Attention: tanh-scored softmax attention pooling.

reference:
    cat = concat([query, embeddings], axis=2)            # (B, L, Q+E)
    h = tanh(cat @ W_attn.T + b_attn)                    # (B, L, E)
    scores = h @ v_w                                     # (B, L)
    w = softmax(scores, axis=1)                          # (B, L)
    applied = (w[:, None, :] @ embeddings)               # (B, 1, E)
    returns (applied, w[:, None, :])

Sharding: data-parallel over batch, 4 batches per core on 8 cores.
Weights (W_attn, b_attn, v_w) replicated.

Per-core dataflow (fp16 PE inputs, fp32 accumulation/everything else):
  - load q/emb natural tiles [128 l, 512 f] fp32, cast to fp16 on-chip (ACT/DVE)
  - PE-transpose 128x128 fp16 blocks into catT chunks, DVE/ACT-copy to sbuf
  - h[l,e] accumulated in psum over 8 f-chunks (lhsT = catT chunk, rhs = W_T chunk)
  - DVE adds bias (b_attn broadcast tile), ACT tanh
  - DVE scalar_tensor_tensor(h * v_bcast, accum=sum over e) -> scores column
    (tensor_tensor_reduce is a custom-DVE op that crashes this runtime)
  - per batch: ACT exp (fp16 out), PE ones-matmul partition-sum, DVE reciprocal,
    PE broadcast of 1/S, DVE scale -> weights; PE transpose (f32) -> DMA out
  - applied = sum_t exp_col_t.T @ emb16_tile_t (fp16 in, fp32 psum), scaled by 1/S
"""

import sys

for _p in ("/opt/trn_rl_repo",):
    if _p not in sys.path:
        sys.path.append(_p)

import numpy as np

import concourse.bass as bass
import concourse.tile as tile
from concourse import bacc, mybir
from concourse.bass_utils import run_bass_kernel_spmd
from concourse.masks import make_identity

F32 = mybir.dt.float32
F32R = mybir.dt.float32r
F16 = mybir.dt.float16

N_CORES = 8
B, L, E, Q = 32, 2048, 512, 512
F = E + Q                      # cat feature dim
B_PC = B // N_CORES            # batches per core
LT = L // 128                  # l-tiles per batch
FC = F // 128                  # 128-wide f chunks in cat
QC = Q // 128                  # f chunks coming from query


def build_nc(b_pc=B_PC, lt=LT, reps=1, timing_R=None):
    """timing_R: if set, embeddings/query become internal DRAM (garbage data,
    no host transfer) and the whole body runs in a hardware For_i loop of
    timing_R iterations — used only to measure per-iteration HW time."""
    nc = bacc.Bacc("TRN2", target_bir_lowering=False, debug=False)

    if timing_R is not None:
        emb_d = nc.dram_tensor("emb_int", [b_pc, lt * 128, E], F32).ap()
        q_d = nc.dram_tensor("q_int", [b_pc, lt * 128, Q], F32).ap()
    else:
        emb_d = nc.dram_tensor("embeddings", [b_pc, lt * 128, E], F32, kind="ExternalInput").ap()
        q_d = nc.dram_tensor("query", [b_pc, lt * 128, Q], F32, kind="ExternalInput").ap()
    wt_d = nc.dram_tensor("w_t", [F, E], F16, kind="ExternalInput").ap()      # W_attn.T
    bias_d = nc.dram_tensor("bias_bc", [128, E], F32, kind="ExternalInput").ap()
    v_d = nc.dram_tensor("v_bc", [128, E], F32, kind="ExternalInput").ap()     # v_w bcast
    id_d = nc.dram_tensor("ident", [128, 128], F16, kind="ExternalInput").ap()
    onec_d = nc.dram_tensor("ones_col", [128, 1], F16, kind="ExternalInput").ap()
    app_d = nc.dram_tensor("applied", [b_pc, 1, E], F32, kind="ExternalOutput").ap()
    wout_d = nc.dram_tensor("weights", [b_pc, 1, lt * 128], F32, kind="ExternalOutput").ap()

    with tile.TileContext(nc) as tc:
        with (
            tc.tile_pool(name="consts", bufs=1) as consts,
            tc.tile_pool(name="embp", bufs=2 * lt) as embp,
            tc.tile_pool(name="qp", bufs=4) as qp,
            tc.tile_pool(name="ctq", bufs=4) as ctqp,
            tc.tile_pool(name="cte", bufs=4) as ctep,
            tc.tile_pool(name="hp", bufs=4) as hp,
            tc.tile_pool(name="batchp", bufs=2) as batchp,
            tc.tile_pool(name="smallp", bufs=4) as smallp,
            tc.tile_pool(name="ps_ct", bufs=4, space="PSUM") as ps_ct,
            tc.tile_pool(name="ps_h", bufs=2, space="PSUM") as ps_h,
            tc.tile_pool(name="ps_sm", bufs=1, space="PSUM") as ps_sm,
        ):
            ident_f32 = consts.tile([128, 128], F32)
            make_identity(nc, ident_f32)
            ones = consts.tile([128, 128], F32)
            nc.vector.memset(ones, 1.0)

            ident_r = consts.tile([128, 128], F16)
            nc.sync.dma_start(out=ident_r, in_=id_d)
            onec_sb = consts.tile([128, 1], F16)
            nc.sync.dma_start(out=onec_sb, in_=onec_d)
            w_sb = consts.tile([128, FC, E], F16)
            nc.sync.dma_start(out=w_sb, in_=wt_d.rearrange("(c p) e -> p c e", p=128))
            bias_sb = consts.tile([128, E], F32)
            nc.sync.dma_start(out=bias_sb, in_=bias_d)
            v_sb = consts.tile([128, E], F32)
            nc.sync.dma_start(out=v_sb, in_=v_d)

            def load_and_transpose(b, t):
                """DMA-load q/emb l-tile t (f32), cast to fp16, PE-transpose
                into catT sbuf chunks; keep the fp16 emb tile for applied."""
                q_tile = qp.tile([128, Q], F32, tag="qf32")
                nc.sync.dma_start(out=q_tile, in_=q_d[b, t * 128:(t + 1) * 128, :])
                e_tile = qp.tile([128, E], F32, tag="ef32")
                nc.sync.dma_start(out=e_tile, in_=emb_d[b, t * 128:(t + 1) * 128, :])
                q16 = qp.tile([128, Q], F16, tag="q16")
                nc.scalar.copy(q16, q_tile)
                e16 = embp.tile([128, E], F16)
                nc.vector.tensor_copy(e16, e_tile)

                pq = ps_ct.tile([128, 4 * 128], F16, tag="psct")
                for c in range(QC):
                    nc.tensor.transpose(
                        pq[:, c * 128:(c + 1) * 128], q16[:, c * 128:(c + 1) * 128], ident_r
                    )
                ctq = ctqp.tile([128, 4 * 128], F16)
                nc.vector.tensor_copy(ctq, pq)

                pe = ps_ct.tile([128, 4 * 128], F16, tag="psct")
                for c in range(FC - QC):
                    nc.tensor.transpose(
                        pe[:, c * 128:(c + 1) * 128], e16[:, c * 128:(c + 1) * 128], ident_r
                    )
                cte = ctep.tile([128, 4 * 128], F16)
                nc.scalar.copy(cte, pe)
                return e16, ctq, cte

            def body():
                # two-tile-ahead software pipeline: transposes/copies run two
                # l-tiles ahead of the matmuls so the PE never waits on a copy.
                order = [(bb, tt) for bb in range(b_pc) for tt in range(lt)]
                staged = {}
                for j in range(min(2, len(order))):
                    staged[j] = load_and_transpose(*order[j])
                idx = 0
                for b in range(b_pc):
                    etiles = []
                    scores_b = batchp.tile([128, lt], F32, tag="scores")
                    for t in range(lt):
                        e_tile, ctq, cte = staged.pop(idx)
                        etiles.append(e_tile)
                        if idx + 2 < len(order):
                            staged[idx + 2] = load_and_transpose(*order[idx + 2])
                        idx += 1

                        ph = ps_h.tile([128, E], F32)
                        for k in range(FC):
                            src = ctq if k < QC else cte
                            c = k if k < QC else k - QC
                            nc.tensor.matmul(
                                ph,
                                src[:, c * 128:(c + 1) * 128],
                                w_sb[:, k, :],
                                start=(k == 0),
                                stop=(k == FC - 1),
                            )

                        hb = hp.tile([128, E], F32, tag="hb")
                        nc.vector.tensor_add(hb, ph, bias_sb)
                        h_sb = hp.tile([128, E], F32, tag="h")
                        nc.scalar.activation(h_sb, hb, mybir.ActivationFunctionType.Tanh)
                        hv = hp.tile([128, E], F32, tag="hv")
                        nc.vector.scalar_tensor_tensor(
                            out=hv,
                            in0=h_sb,
                            scalar=1.0,
                            in1=v_sb,
                            op0=mybir.AluOpType.mult,
                            op1=mybir.AluOpType.mult,
                            accum_out=scores_b[:, t:t + 1],
                        )

                    # --- batch tail: softmax + outputs ---
                    exp_b = batchp.tile([128, lt], F16, tag="exp")
                    nc.scalar.activation(exp_b, scores_b, mybir.ActivationFunctionType.Exp)

                    ps_s = ps_sm.tile([1, lt], F32, tag="pssm")
                    nc.tensor.matmul(ps_s, onec_sb, exp_b)  # partition sum -> [1, lt]
                    s_sb = smallp.tile([1, lt], F32, tag="s")
                    nc.vector.tensor_copy(s_sb, ps_s)
                    tot = smallp.tile([1, 1], F32, tag="tot")
                    nc.vector.reduce_sum(tot, s_sb, axis=mybir.AxisListType.X)
                    rcp = smallp.tile([1, 1], F32, tag="rcp")
                    nc.vector.reciprocal(rcp, tot)

                    ps_rb = ps_sm.tile([128, 1], F32, tag="pssm")
                    nc.tensor.matmul(ps_rb, ones[0:1, :], rcp)  # bcast 1/S to 128 parts
                    rb_sb = smallp.tile([128, 1], F32, tag="rb")
                    nc.vector.tensor_copy(rb_sb, ps_rb)

                    wgt = batchp.tile([128, lt], F32, tag="wgt")
                    nc.vector.tensor_scalar_mul(wgt, exp_b, rb_sb)
                    ps_wt = ps_sm.tile([lt, 128], F32, tag="pssm")
                    nc.tensor.transpose(ps_wt, wgt, ident_f32)
                    wt_sb = smallp.tile([lt, 128], F32, tag="wT")
                    nc.vector.tensor_copy(wt_sb, ps_wt)
                    nc.sync.dma_start(
                        out=wout_d[b, 0, :].rearrange("(t p) -> t p", p=128), in_=wt_sb
                    )

                    ps_a = ps_sm.tile([1, E], F32, tag="psa")
                    for t in range(lt):
                        nc.tensor.matmul(
                            ps_a,
                            exp_b[:, t:t + 1],
                            etiles[t],
                            start=(t == 0),
                            stop=(t == lt - 1),
                        )
                    app_sb = smallp.tile([1, E], F32, tag="app")
                    nc.vector.tensor_scalar_mul(app_sb, ps_a, rcp)
                    nc.sync.dma_start(out=app_d[b, 0:1, :], in_=app_sb)

            if timing_R is not None:
                with tc.For_i(0, timing_R, 1):
                    body()
            else:
                for _ in range(reps):
                    body()

    nc.finalize()
    return nc


_NC_CACHE = {}


def _get_nc(b_pc=B_PC, lt=LT, reps=1, timing_R=None):
    key = (b_pc, lt, reps, timing_R)
    if key not in _NC_CACHE:
        _NC_CACHE[key] = build_nc(*key)
    return _NC_CACHE[key]


def _prep_shared(W_attn, b_attn, v_w):
    return {
        "w_t": np.ascontiguousarray(W_attn.T).astype(np.float16),
        "bias_bc": np.ascontiguousarray(np.broadcast_to(b_attn, (128, E))),
        "v_bc": np.ascontiguousarray(np.broadcast_to(v_w, (128, E))),
        "ident": np.eye(128, dtype=np.float16),
        "ones_col": np.ones((128, 1), dtype=np.float16),
    }


def kernel(embeddings, query, W_attn, b_attn, v_w):
    embeddings = np.asarray(embeddings, dtype=np.float32)
    query = np.asarray(query, dtype=np.float32)
    shared = _prep_shared(np.asarray(W_attn, np.float32), np.asarray(b_attn, np.float32),
                          np.asarray(v_w, np.float32))
    nc = _get_nc()
    in_maps = []
    for i in range(N_CORES):
        sl = slice(i * B_PC, (i + 1) * B_PC)
        in_maps.append({"embeddings": embeddings[sl], "query": query[sl], **shared})
    res = run_bass_kernel_spmd(nc, in_maps, list(range(N_CORES)))
    applied = np.concatenate([res.results[i]["applied"] for i in range(N_CORES)], axis=0)
    weights = np.concatenate([res.results[i]["weights"] for i in range(N_CORES)], axis=0)
    return applied, weights
